# revision 1
# baseline (speedup 1.0000x reference)
"""Bass/Tile Trainium2 kernel for the 2-layer FC-LSTM + Dense model.

Strategy (data-parallel over batch, 8 cores x 32 samples):
  - All on-chip activations live in "transposed" layout: feature dim on the
    128 SBUF partitions, (time x batch) on the free dim, so elementwise ops
    run with all 128 lanes active and the recurrent matmuls keep the gate
    weights as the PE-stationary operand.
  - Gate columns of W/U are host-permuted from keras order [i f g o] to
    [i f o g] so one sigmoid covers a contiguous 6-chunk block.
  - b0 is folded into the input GEMM via a ones-row in the K-padded input
    (900 -> 1024, row 900 = 1.0, W0 row 900 = b0); b1/bd are applied as
    per-partition bias APs on the ScalarE psum-evacuation copy.
  - Matmuls in bf16 (fp32 PSUM accumulation); cell state and gate math fp32.
"""

import numpy as np
import ml_dtypes
from contextlib import ExitStack

import concourse.bass as bass
import concourse.mybir as mybir
import concourse.tile as tile
from concourse import bacc, bass_utils

# problem constants (hardcoded per contract)
B, N, T, F_IN = 256, 300, 64, 3
U_DIM = 256
G = 4 * U_DIM              # 1024 gates per layer
F_OUT = 2
D_IN = N * F_IN            # 900
D_OUT = N * F_OUT          # 600
NCORES = 8
BL = B // NCORES           # 32 batch rows per core
NTOK = T * BL              # 2048 tokens per core (token id = t*BL + b)
KP = 1024                  # padded input-feature dim; row 900 is the bias row
KT_IN = KP // 128          # 8 k-tiles for the input GEMM
GC = G // 128              # 8 gate chunks
HC = U_DIM // 128          # 2 hidden chunks
MT = 5                     # output m-tiles (600 -> 640)
D_OUT_PAD = MT * 128
CHUNK = 512                # token chunk in GEMM phases
NCH = NTOK // CHUNK        # 4
TPC = CHUNK // BL          # 16 timesteps per token chunk

BF16 = mybir.dt.bfloat16
F32 = mybir.dt.float32
NP_BF16 = ml_dtypes.bfloat16
AF = mybir.ActivationFunctionType
ALU = mybir.AluOpType

# keras gate order i,f,g,o -> our chunk order i,f,o,g
_PERM = np.concatenate([
    np.arange(0, U_DIM),                  # i
    np.arange(U_DIM, 2 * U_DIM),          # f
    np.arange(3 * U_DIM, 4 * U_DIM),      # o
    np.arange(2 * U_DIM, 3 * U_DIM),      # g
])


def _w_tiles(Wp, kt):
    """(kt*128, GC*128) f32 -> (128, kt, GC, 128) bf16 lhsT tile array."""
    return np.ascontiguousarray(
        Wp.astype(NP_BF16).reshape(kt, 128, GC, 128).transpose(1, 0, 2, 3))


def _prep_shared(W0, U0, b0, W1, U1, b1, Wd, bd):
    W0p = np.zeros((KP, G), np.float32)
    W0p[:D_IN] = W0[:, _PERM]
    W0p[D_IN] = b0[_PERM]
    w0t = _w_tiles(W0p, KT_IN)
    u0t = _w_tiles(U0[:, _PERM], HC)
    w1t = _w_tiles(W1[:, _PERM], HC)
    u1t = _w_tiles(U1[:, _PERM], HC)
    Wdp = np.zeros((U_DIM, D_OUT_PAD), np.float32)
    Wdp[:, :D_OUT] = Wd
    wdt = np.ascontiguousarray(
        Wdp.astype(NP_BF16).reshape(HC, 128, MT, 128).transpose(1, 0, 2, 3))
    b1mm = np.zeros((128, GC, 128), np.float32)
    b1mm[D_IN % 128] = b1[_PERM].reshape(GC, 128)   # rank-1 bias vs xT's ones row
    b1mm = b1mm.astype(NP_BF16)
    bdp = np.zeros(D_OUT_PAD, np.float32)
    bdp[:D_OUT] = bd
    bdt = np.ascontiguousarray(bdp.reshape(MT, 128).T)
    return dict(w0t=w0t, u0t=u0t, w1t=w1t, u1t=u1t, wdt=wdt, b1mm=b1mm, bdt=bdt)


def _prep_x(x_core):
    """(BL, N, T, F_IN) f32 -> (128, KT_IN, NTOK) bf16 with bias ones-row."""
    seq = x_core.transpose(0, 2, 1, 3).reshape(BL, T, D_IN)   # (b, t, feat)
    xT = np.zeros((KP, T, BL), np.float32)
    xT[:D_IN] = seq.transpose(2, 1, 0)                        # (feat, t, b)
    xT[D_IN] = 1.0
    return np.ascontiguousarray(
        xT.astype(NP_BF16).reshape(KT_IN, 128, NTOK).transpose(1, 0, 2))


class _LstmStepper:
    """Per-step emitters for one LSTM layer, split into sub-phases so the
    build loop can interleave two layers' instruction streams per engine.

    z(t) = zx(t) + U h(t-1) is built entirely in PSUM: 8 identity matmuls
    prefill the bank with zx(t) (exact in bf16, one shared Ldweights after
    dedup, and off the serial chain since zx is ready long in advance), then
    the 16 U matmuls accumulate on top. The sigmoid and the relu(g)*i fused
    op read PSUM directly -- no z materialization, no DVE z-add, two fewer
    cross-engine hops on the per-step critical path."""

    def __init__(self, nc, work, ps_r, u, zx, hseq, cst, lname, use_gpsimd,
                 ident, w_in=None, hprev=None, bias_mm=None, ones_row=None):
        self.nc, self.work, self.ps_r = nc, work, ps_r
        self.u, self.zx, self.hseq, self.cst = u, zx, hseq, cst
        self.ln = lname
        self.use_gpsimd = use_gpsimd
        self.ident = ident
        # fused input path: z(t) = W_in.T hprev(t) + b (rank-1 matmul) + U h(t-1)
        self.w_in, self.hprev = w_in, hprev
        self.bias_mm, self.ones_row = bias_mm, ones_row
        self.ps = self.sfo = None

    def emit_mm(self, t):
        nc = self.nc
        ps = self.ps_r.tile([128, GC, BL], F32, tag=f"{self.ln}ps",
                            name=f"{self.ln}_ps_{t}", bufs=2)
        self.ps = ps
        last_in = t == 0
        if self.w_in is None:
            # prefill with precomputed zx(t) via identity matmuls (one shared
            # Ldweights after dedup; exact in bf16)
            for g in range(GC):
                nc.tensor.matmul(
                    ps[:, g, :], self.ident[:], self.zx[:, t, g, :],
                    start=(g == 0), stop=(g == GC - 1 and last_in))
        else:
            # fused: z(t) = W_in.T hprev(t) (+ bias as rank-1 vs the ones row)
            for g in range(GC):
                for k in range(HC):
                    nc.tensor.matmul(
                        ps[:, g, :], self.w_in[:, k, g, :], self.hprev[:, t, k, :],
                        start=(g == 0 and k == 0), stop=False)
                nc.tensor.matmul(
                    ps[:, g, :], self.bias_mm[:, g, :], self.ones_row(t),
                    start=False, stop=(g == GC - 1 and last_in))
        if t == 0:
            return
        for g in range(GC):
            for k in range(HC):
                nc.tensor.matmul(
                    ps[:, g, :], self.u[:, k, g, :], self.hseq[:, t - 1, k, :],
                    start=False, stop=(g == GC - 1 and k == HC - 1))

    def emit_sig(self, t):
        sfo = self.work.tile([128, 6, BL], F32, tag=f"{self.ln}sfo",
                             name=f"{self.ln}_sfo_{t}")
        self.nc.scalar.activation(sfo[:], self.ps[:, 0:6, :], AF.Sigmoid)
        self.sfo = sfo

    def emit_cell(self, t):
        nc, sfo, cst = self.nc, self.sfo, self.cst
        ew = nc.gpsimd if self.use_gpsimd else nc.vector
        if t == 0:
            nc.vector.scalar_tensor_tensor(
                cst[:], self.ps[:, 6:8, :], 0.0, sfo[:, 0:2, :],
                op0=ALU.max, op1=ALU.mult)
        else:
            t1 = self.work.tile([128, 2, BL], F32, tag=f"{self.ln}t1",
                                name=f"{self.ln}_t1_{t}")
            nc.vector.scalar_tensor_tensor(
                t1[:], self.ps[:, 6:8, :], 0.0, sfo[:, 0:2, :],
                op0=ALU.max, op1=ALU.mult)
            ew.tensor_mul(cst[:], cst[:], sfo[:, 2:4, :])
            ew.tensor_add(cst[:], cst[:], t1[:])
        nc.vector.scalar_tensor_tensor(
            self.hseq[:, t, :, :], cst[:], 0.0, sfo[:, 4:6, :],
            op0=ALU.max, op1=ALU.mult)


def build_nc(p1=True, rec0=True, p3=True, rec1=True, p5=True, rec_t=T,
             wavefront=True, use_gpsimd=False):
    nc = bacc.Bacc("TRN2", target_bir_lowering=False, debug=False)
    xT_d = nc.dram_tensor("xT", (128, KT_IN, NTOK), BF16, kind="ExternalInput").ap()
    w0_d = nc.dram_tensor("w0t", (128, KT_IN, GC, 128), BF16, kind="ExternalInput").ap()
    u0_d = nc.dram_tensor("u0t", (128, HC, GC, 128), BF16, kind="ExternalInput").ap()
    w1_d = nc.dram_tensor("w1t", (128, HC, GC, 128), BF16, kind="ExternalInput").ap()
    u1_d = nc.dram_tensor("u1t", (128, HC, GC, 128), BF16, kind="ExternalInput").ap()
    wd_d = nc.dram_tensor("wdt", (128, HC, MT, 128), BF16, kind="ExternalInput").ap()
    b1_d = nc.dram_tensor("b1mm", (128, GC, 128), BF16, kind="ExternalInput").ap()
    id_d = nc.dram_tensor("ident", (128, 128), BF16, kind="ExternalInput").ap()
    bd_d = nc.dram_tensor("bdt", (128, MT), F32, kind="ExternalInput").ap()
    out_d = nc.dram_tensor("out", (128, MT, NTOK), F32, kind="ExternalOutput").ap()

    with tile.TileContext(nc) as tc, ExitStack() as ctx:
        const = ctx.enter_context(tc.tile_pool(name="const", bufs=1))
        xT = const.tile([128, KT_IN, NTOK], BF16)
        w0 = const.tile([128, KT_IN, GC, 128], BF16)
        u0 = const.tile([128, HC, GC, 128], BF16)
        w1 = const.tile([128, HC, GC, 128], BF16)
        u1 = const.tile([128, HC, GC, 128], BF16)
        wd = const.tile([128, HC, MT, 128], BF16)
        b1mm = const.tile([128, GC, 128], BF16)
        ident = const.tile([128, 128], BF16)
        bds = const.tile([128, MT], F32)
        zx0 = const.tile([128, T, GC, BL], BF16)
        h0 = const.tile([128, T, HC, BL], BF16)
        h1 = const.tile([128, T, HC, BL], BF16)
        c0 = const.tile([128, HC, BL], F32)
        c1 = const.tile([128, HC, BL], F32)

        # need-ordered input DMA: chunk-0 inputs first (so the first zx0 GEMM
        # groups start a few us in), then recurrence/L1 weights, then the rest
        # of the sequence streaming under the round loop. Transfers alternate
        # between the two HWDGE issue engines (SP, ACT) to run on two queues,
        # and the bulk tail goes through gpsimd's SWDGE queues.
        hw_eng = [nc.sync, nc.scalar]
        for k in range(KT_IN):
            hw_eng[k % 2].dma_start(out=xT[:, k, 0:CHUNK], in_=xT_d[:, k, 0:CHUNK])
        for g in range(GC):
            hw_eng[g % 2].dma_start(out=w0[:, :, g, :], in_=w0_d[:, :, g, :])
        for i, (sb, dr) in enumerate(((ident, id_d), (u0, u0_d), (w1, w1_d),
                                      (u1, u1_d), (b1mm, b1_d))):
            hw_eng[i % 2].dma_start(out=sb[:], in_=dr[:])
        for cix in range(1, NCH):
            for k in range(KT_IN):
                nc.gpsimd.dma_start(
                    out=xT[:, k, cix * CHUNK:(cix + 1) * CHUNK],
                    in_=xT_d[:, k, cix * CHUNK:(cix + 1) * CHUNK])
        for sb, dr in ((wd, wd_d), (bds, bd_d)):
            nc.gpsimd.dma_start(out=sb[:], in_=dr[:])

        ps_g = ctx.enter_context(tc.tile_pool(name="ps_g", bufs=2, space="PSUM"))
        ps_r = ctx.enter_context(tc.tile_pool(name="ps_r", bufs=2, space="PSUM"))
        work = ctx.enter_context(tc.tile_pool(name="work", bufs=3))
        outp = ctx.enter_context(tc.tile_pool(name="outp", bufs=3))

        def p1_g_group(cix, g):
            # zx0[:, chunk, g] = W0p[:, g].T @ xT (bias via ones-row), bf16 evac
            ps = ps_g.tile([128, CHUNK], F32, tag="psg", name=f"p1_{g}_{cix}")
            for k in range(KT_IN):
                nc.tensor.matmul(
                    ps[:], w0[:, k, g, :], xT[:, k, cix * CHUNK:(cix + 1) * CHUNK],
                    start=(k == 0), stop=(k == KT_IN - 1))
            nc.scalar.copy(
                zx0[:, cix * TPC:(cix + 1) * TPC, g, :],
                ps.rearrange("p (t b) -> p t b", b=BL))

        def p1_chunk(cix):
            for g in range(GC):
                p1_g_group(cix, g)

        def p5_chunk(cix):
            # out[:, chunk] = Wd.T @ h1 + bd
            for m in range(MT):
                ps = ps_g.tile([128, CHUNK], F32, tag="psg", name=f"p5_{m}_{cix}")
                for k in range(HC):
                    nc.tensor.matmul(
                        ps[:], wd[:, k, m, :], h1[:, cix * TPC:(cix + 1) * TPC, k, :],
                        start=(k == 0), stop=(k == HC - 1))
                ot = outp.tile([128, CHUNK], F32, tag="ot", name=f"ot{m}_{cix}")
                nc.scalar.activation(ot[:], ps[:], AF.Identity, bias=bds[:, m:m + 1])
                nc.sync.dma_start(
                    out=out_d[:, m, cix * CHUNK:(cix + 1) * CHUNK], in_=ot[:])

        st0 = _LstmStepper(nc, work, ps_r, u0, zx0, h0, c0, "l0", use_gpsimd, ident)
        st1 = _LstmStepper(nc, work, ps_r, u1, None, h1, c1, "l1", use_gpsimd, ident,
                           w_in=w1, hprev=h0, bias_mm=b1mm,
                           ones_row=lambda t: xT[:, KT_IN - 1, t * BL:(t + 1) * BL])

        def full_step(st, t):
            st.emit_mm(t)
            st.emit_sig(t)
            st.emit_cell(t)

        if not wavefront:
            for cix in range(NCH if p1 else 0):
                p1_chunk(cix)
            if rec0:
                for t in range(rec_t):
                    full_step(st0, t)
            if rec1:
                for t in range(rec_t):
                    full_step(st1, t)
            for cix in range(NCH if p5 else 0):
                p5_chunk(cix)
        else:
            # Wavefront: L1 trails L0 by LAG steps. Per round, the two layers'
            # matmul groups are back-to-back on the PE, and their gate tails
            # are interleaved at op granularity so ScalarE's sigmoid for one
            # layer runs under VectorE's z-add for the other -- the in-order
            # per-engine streams then pipeline instead of blocking on the
            # cross-engine hop. zx1/out chunk GEMMs fire as soon as the 16
            # timesteps they read are complete.
            LAG = 2
            p1_chunk(0)
            for t in range(T + LAG):
                j = t - LAG
                a = t if t < T else None
                b = j if 0 <= j < T else None
                if a is not None:
                    st0.emit_mm(a)
                if b is not None:
                    st1.emit_mm(b)
                # spread the remaining input-GEMM groups through the rounds so
                # the in-order PE stream fills chain gaps instead of front-
                # running the recurrence: one g-group every other round
                # finishes chunk c+1 just before round 16(c+1) needs it.
                if a is not None and a < 48 and a % 2 == 0:
                    p1_g_group(a // TPC + 1, (a % TPC) // 2)
                if a is not None:
                    st0.emit_sig(a)
                if b is not None:
                    st1.emit_sig(b)
                if a is not None:
                    st0.emit_cell(a)
                if b is not None:
                    st1.emit_cell(b)
                if b is not None and b % TPC == TPC - 1:
                    p5_chunk(b // TPC)
    ndup = _dedup_ldweights(nc)
    nc.compile()
    return nc


def _dedup_ldweights(nc):
    """Drop an Ldweights whose stationary operand is identical to the
    immediately-preceding PE weight load -- the array still holds it, so the
    following (non-self-loading) Matmult reuses the loaded weights."""
    removed = 0
    for blk in nc.m.functions[0].blocks:
        insts = blk.instructions
        out = []
        last_w = None
        for inst in insts:
            if inst.opcode == "Ldweights":
                si = inst.sync_info
                key = repr(inst.ins[0])
                if key == last_w and (si is None or not si.on_wait):
                    removed += 1
                    continue
                last_w = key
            elif inst.opcode != "Matmult" and inst.engine == mybir.EngineType.PE:
                last_w = None
            out.append(inst)
        if removed:
            blk.instructions.clear()
            blk.instructions.extend(out)
    return removed


_IDENT = np.ascontiguousarray(np.eye(128, dtype=np.float32).astype(NP_BF16))
_NC_CACHE = {}
LAST_RESULTS = []  # test harness introspection (exec_time_ns / traces)


def _get_nc():
    if "nc" not in _NC_CACHE:
        _NC_CACHE["nc"] = build_nc()
    return _NC_CACHE["nc"]


def kernel(**inputs):
    x = np.asarray(inputs["x"], np.float32)
    shared = _prep_shared(
        np.asarray(inputs["W0"], np.float32), np.asarray(inputs["U0"], np.float32),
        np.asarray(inputs["b0"], np.float32), np.asarray(inputs["W1"], np.float32),
        np.asarray(inputs["U1"], np.float32), np.asarray(inputs["b1"], np.float32),
        np.asarray(inputs["Wd"], np.float32), np.asarray(inputs["bd"], np.float32))
    in_maps = []
    for c in range(NCORES):
        m = dict(shared)
        m["xT"] = _prep_x(x[c * BL:(c + 1) * BL])
        m["ident"] = _IDENT
        in_maps.append(m)

    nc = _get_nc()
    res = bass_utils.run_bass_kernel_spmd(nc, in_maps, core_ids=list(range(NCORES)))
    LAST_RESULTS.append(res)

    outs = []
    for c in range(NCORES):
        o = np.asarray(res.results[c]["out"], np.float32)      # (128, MT, NTOK)
        yT = o.transpose(1, 0, 2).reshape(D_OUT_PAD, NTOK)[:D_OUT]
        y = yT.T.reshape(T, BL, N, F_OUT).transpose(1, 2, 0, 3)
        outs.append(y)
    return np.ascontiguousarray(np.concatenate(outs, axis=0), dtype=np.float32)



# revision 28
# speedup vs baseline: 1.2037x; 1.2037x over previous
"""Bass/Tile Trainium2 kernel for the 2-layer FC-LSTM + Dense model.

Strategy (data-parallel over batch, 8 cores x 32 samples):
  - All on-chip activations live in "transposed" layout: feature dim on the
    128 SBUF partitions, (time x batch) on the free dim.
  - Gate columns of W/U are host-permuted from keras order [i f g o] to
    [i f o g] so one sigmoid covers a contiguous 6-chunk block.
  - The input GEMM writes z0 = W0.T x + b0 DIRECTLY into the recurrence
    PSUM banks in 4-step "quarters" (double-buffered, 2 banks each); the
    U matmuls then accumulate on top and the gate math reads PSUM in
    place. No identity prefill, no PSUM evacuation, no zx staging in SBUF.
  - b0 is folded into the input GEMM via a ones-row in the K-padded input.
    b1/bd keep a generic path, but when they are all-zero (as produced by
    setup_inputs) a fast path drops the L1 bias matmul.
  - One p1 quarter-slice (16 matmuls) is injected per recurrence round so
    there is no serial GEMM prologue; PE is warmed with dummy matmuls
    during the initial weight DMA; the first ACT op is a sigmoid so a
    single table-set load covers every later activation/copy.
  - c(t) = f*c + i*relu(g) is nonnegative by induction (c0=0), so
    h = o*relu(c) == o*c and the h-write is a plain tensor_tensor multiply.
  - Matmuls in bf16 (fp32 PSUM accumulation); cell state and gate math fp32.
"""

import numpy as np
import ml_dtypes
from contextlib import ExitStack

import concourse.bass as bass
import concourse.mybir as mybir
import concourse.tile as tile
from concourse.tile_rust import add_dep_helper
from concourse import bacc, bass_utils

# problem constants (hardcoded per contract)
B, N, T, F_IN = 256, 300, 64, 3
U_DIM = 256
G = 4 * U_DIM              # 1024 gates per layer
F_OUT = 2
D_IN = N * F_IN            # 900
D_OUT = N * F_OUT          # 600
NCORES = 8
BL = B // NCORES           # 32 batch rows per core
NTOK = T * BL              # 2048 tokens per core (token id = t*BL + b)
KP = 1024                  # padded input-feature dim; row 900 is the bias row
KT_IN = KP // 128          # 8 k-tiles for the input GEMM
GC = G // 128              # 8 gate chunks
HC = U_DIM // 128          # 2 hidden chunks
MT = 5                     # output m-tiles (600 -> 640)
D_OUT_PAD = MT * 128
QS = 1                     # steps per z0 PSUM bank (one group per bank)
QTOK = QS * BL             # 32 tokens per step-bank
NQ = T // QS               # one z0 bank per step
PCH = 8                    # steps per output-GEMM chunk
PTOK = PCH * BL            # 256 tokens
NPCH = T // PCH            # 8 chunks
NWARM = 16                 # PE warm-up matmuls (N=256)

BF16 = mybir.dt.bfloat16
F32 = mybir.dt.float32
NP_BF16 = ml_dtypes.bfloat16
AF = mybir.ActivationFunctionType
ALU = mybir.AluOpType

# keras gate order i,f,g,o -> our chunk order i,f,o,g
_PERM = np.concatenate([
    np.arange(0, U_DIM),                  # i
    np.arange(U_DIM, 2 * U_DIM),          # f
    np.arange(3 * U_DIM, 4 * U_DIM),      # o
    np.arange(2 * U_DIM, 3 * U_DIM),      # g
])


def _w_tiles(Wp, kt):
    """(kt*128, GC*128) f32 -> (128, kt, GC, 128) bf16 lhsT tile array."""
    return np.ascontiguousarray(
        Wp.astype(NP_BF16).reshape(kt, 128, GC, 128).transpose(1, 0, 2, 3))


def _prep_shared(W0, U0, b0, W1, U1, b1, Wd, bd):
    W0p = np.zeros((KP, G), np.float32)
    W0p[:D_IN] = W0[:, _PERM]
    W0p[D_IN] = b0[_PERM]
    w0t = _w_tiles(W0p, KT_IN)
    u0t = _w_tiles(U0[:, _PERM], HC)
    w1t = _w_tiles(W1[:, _PERM], HC)
    u1t = _w_tiles(U1[:, _PERM], HC)
    Wdp = np.zeros((U_DIM, D_OUT_PAD), np.float32)
    Wdp[:, :D_OUT] = Wd
    wdt = np.ascontiguousarray(
        Wdp.astype(NP_BF16).reshape(HC, 128, MT, 128).transpose(1, 0, 2, 3))
    # L1 bias via one matmul (generic path): stationary rows 0..7 hold b1
    # reshaped (8, 128); the moving "indicator" is 1.0 at (row g, col-block g).
    b1sq = np.zeros((128, 128), np.float32)
    b1sq[:GC] = b1[_PERM].reshape(GC, 128)
    b1sq = b1sq.astype(NP_BF16)
    indic = np.zeros((128, GC, BL), np.float32)
    for g in range(GC):
        indic[g, g, :] = 1.0
    indic = np.ascontiguousarray(indic.reshape(128, GC * BL).astype(NP_BF16))
    bdp = np.zeros(D_OUT_PAD, np.float32)
    bdp[:D_OUT] = bd
    bdt = np.ascontiguousarray(bdp.reshape(MT, 128).T)
    return dict(w0t=w0t, u0t=u0t, w1t=w1t, u1t=u1t, wdt=wdt, b1sq=b1sq,
                indic=indic, bdt=bdt)


def _prep_x(x_core):
    """(BL, N, T, F_IN) f32 -> (128, KT_IN, NTOK) bf16 with bias ones-row."""
    seq = x_core.transpose(0, 2, 1, 3).reshape(BL, T, D_IN)   # (b, t, feat)
    xT = np.zeros((KP, T, BL), np.float32)
    xT[:D_IN] = seq.transpose(2, 1, 0)                        # (feat, t, b)
    xT[D_IN] = 1.0
    return np.ascontiguousarray(
        xT.astype(NP_BF16).reshape(KT_IN, 128, NTOK).transpose(1, 0, 2))


class _L0Stepper:
    """L0: z lives in the p1 quarter PSUM tile; U matmuls accumulate in
    place and the gate math reads the step's slice directly."""

    def __init__(self, nc, work, u, hseq, cst):
        self.nc, self.work = nc, work
        self.u, self.hseq, self.cst = u, hseq, cst
        self.zq = None     # current quarter tile, set by the build loop
        self.ps = None     # [128, GC, BL] view of step slice
        self.sfo = None

    def set_quarter(self, zq):
        self.zq = zq

    def emit_mm(self, t):
        nc = self.nc
        self.ps = self.zq[:, :, 0, :]
        if t == 0:
            return
        for g in (6, 7, 0, 1, 2, 3, 4, 5):
            for k in range(HC):
                # the bank's accumulation group was opened by the step's
                # first p1 matmul; the last U matmul closes it
                nc.tensor.matmul(
                    self.zq[:, g, 0, :], self.u[:, k, g, :],
                    self.hseq[:, t - 1, k, :],
                    start=False, stop=(g == 5 and k == HC - 1))

    def emit_sig(self, t):
        sfo = self.work.tile([128, 6, BL], BF16, tag="l0sfo", name=f"l0_sfo_{t}")
        self.nc.scalar.activation(sfo[:], self.ps[:, 0:6, :], AF.Sigmoid)
        self.sfo = sfo

    def emit_cell(self, t):
        nc, sfo, cst = self.nc, self.sfo, self.cst
        if t == 0:
            nc.vector.scalar_tensor_tensor(
                cst[:], self.ps[:, 6:8, :], 0.0, sfo[:, 0:2, :],
                op0=ALU.max, op1=ALU.mult)
        else:
            t1 = self.work.tile([128, 2, BL], BF16, tag="l0t1", name=f"l0_t1_{t}")
            nc.vector.scalar_tensor_tensor(
                t1[:], self.ps[:, 6:8, :], 0.0, sfo[:, 0:2, :],
                op0=ALU.max, op1=ALU.mult)
            nc.vector.tensor_mul(cst[:], cst[:], sfo[:, 2:4, :])
            nc.vector.tensor_add(cst[:], cst[:], t1[:])
        # c >= 0 by induction, so h = o * relu(c) == o * c
        return nc.vector.tensor_mul(self.hseq[:, t, :, :], cst[:], sfo[:, 4:6, :])


class _L1Stepper:
    """L1: z(t) = [b1 +] W1.T h0(t) + U1.T h1(t-1), built per step in its
    own PSUM bank."""

    def __init__(self, nc, work, ps_r, u, w_in, hprev, hseq, cst,
                 b1sq=None, indic=None):
        self.nc, self.work, self.ps_r = nc, work, ps_r
        self.u, self.w_in, self.hprev = u, w_in, hprev
        self.hseq, self.cst = hseq, cst
        self.b1sq, self.indic = b1sq, indic
        self.ps = self.sfo = None

    def emit_mm(self, t):
        nc = self.nc
        ps = self.ps_r.tile([128, GC, BL], F32, tag="l1ps",
                            name=f"l1_ps_{t}", bufs=2)
        self.ps = ps
        last_in = t == 0
        if self.b1sq is not None:
            nc.tensor.matmul(ps[:], self.b1sq[:], self.indic[:],
                             start=True, stop=False)
        for g in range(GC):
            for k in range(HC):
                nc.tensor.matmul(
                    ps[:, g, :], self.w_in[:, k, g, :], self.hprev[:, t, k, :],
                    start=(self.b1sq is None and g == 0 and k == 0),
                    stop=(g == GC - 1 and k == HC - 1 and last_in))
        if t == 0:
            return
        for g in (6, 7, 0, 1, 2, 3, 4, 5):
            for k in range(HC):
                nc.tensor.matmul(
                    ps[:, g, :], self.u[:, k, g, :], self.hseq[:, t - 1, k, :],
                    start=False, stop=(g == 5 and k == HC - 1))

    def emit_sig(self, t):
        sfo = self.work.tile([128, 6, BL], BF16, tag="l1sfo", name=f"l1_sfo_{t}")
        self.nc.scalar.activation(sfo[:], self.ps[:, 0:6, :], AF.Sigmoid)
        self.sfo = sfo

    def emit_cell(self, t):
        nc, sfo, cst = self.nc, self.sfo, self.cst
        if t == 0:
            nc.vector.scalar_tensor_tensor(
                cst[:], self.ps[:, 6:8, :], 0.0, sfo[:, 0:2, :],
                op0=ALU.max, op1=ALU.mult)
        else:
            t1 = self.work.tile([128, 2, BL], BF16, tag="l1t1", name=f"l1_t1_{t}")
            nc.vector.scalar_tensor_tensor(
                t1[:], self.ps[:, 6:8, :], 0.0, sfo[:, 0:2, :],
                op0=ALU.max, op1=ALU.mult)
            nc.vector.tensor_mul(cst[:], cst[:], sfo[:, 2:4, :])
            nc.vector.tensor_add(cst[:], cst[:], t1[:])
        return nc.vector.tensor_mul(self.hseq[:, t, :, :], cst[:], sfo[:, 4:6, :])


def build_nc(with_b1=False):
    nc = bacc.Bacc("TRN2", target_bir_lowering=False, debug=False)
    xT_d = nc.dram_tensor("xT", (128, KT_IN, NTOK), BF16, kind="ExternalInput").ap()
    w0_d = nc.dram_tensor("w0t", (128, KT_IN, GC, 128), BF16, kind="ExternalInput").ap()
    u0_d = nc.dram_tensor("u0t", (128, HC, GC, 128), BF16, kind="ExternalInput").ap()
    w1_d = nc.dram_tensor("w1t", (128, HC, GC, 128), BF16, kind="ExternalInput").ap()
    u1_d = nc.dram_tensor("u1t", (128, HC, GC, 128), BF16, kind="ExternalInput").ap()
    wd_d = nc.dram_tensor("wdt", (128, HC, MT, 128), BF16, kind="ExternalInput").ap()
    b1_d = nc.dram_tensor("b1sq", (128, 128), BF16, kind="ExternalInput").ap()
    in_d = nc.dram_tensor("indic", (128, GC * BL), BF16, kind="ExternalInput").ap()
    bd_d = nc.dram_tensor("bdt", (128, MT), F32, kind="ExternalInput").ap()
    out_d = nc.dram_tensor("out", (128, MT, NTOK), F32, kind="ExternalOutput").ap()

    with tile.TileContext(nc) as tc, ExitStack() as ctx:
        const = ctx.enter_context(tc.tile_pool(name="const", bufs=1))
        xT = const.tile([128, KT_IN, NTOK], BF16)
        w0 = const.tile([128, KT_IN, GC, 128], BF16)
        u0 = const.tile([128, HC, GC, 128], BF16)
        w1 = const.tile([128, HC, GC, 128], BF16)
        u1 = const.tile([128, HC, GC, 128], BF16)
        wd = const.tile([128, HC, MT, 128], BF16)
        bds = const.tile([128, MT], F32)
        h0 = const.tile([128, T, HC, BL], BF16)
        h1 = const.tile([128, T, HC, BL], BF16)
        c0 = const.tile([128, HC, BL], BF16)
        c1 = const.tile([128, HC, BL], BF16)
        warm = const.tile([128, 2, 128], BF16)
        junk = const.tile([128, 8], F32)
        if with_b1:
            b1sq = const.tile([128, 128], BF16)
            indic = const.tile([128, GC * BL], BF16)

        # PSUM: z0 step-banks and p5 share one 6-slot pool; L1 z 2 -> 8 banks
        zqp = ctx.enter_context(tc.tile_pool(name="zqp", bufs=6, space="PSUM"))
        ps_r = ctx.enter_context(tc.tile_pool(name="ps_r", bufs=2, space="PSUM"))
        p5p = zqp
        work = ctx.enter_context(tc.tile_pool(name="work", bufs=3))
        outp = ctx.enter_context(tc.tile_pool(name="outp", bufs=3))

        # --- prologue: ACT table preload, PE warm-up, need-ordered DMA ---
        nc.gpsimd.memset(warm[:], 0)
        nc.gpsimd.memset(junk[:], 0)
        # first ACT op is a Sigmoid: the single table-set load covers the
        # later identity copies too (sigmoid_and_others has copy/relu).
        nc.scalar.activation(junk[:], junk[:], AF.Sigmoid)
        # dummy matmuls ramp the PE p-state while the first DMAs land
        wps = p5p.tile([128, 256], F32, tag="zq", name="warmps")
        for i in range(NWARM):
            nc.tensor.matmul(wps[:], warm[:, 0, :], warm.rearrange("p a b -> p (a b)"),
                             start=(i == 0), stop=(i == NWARM - 1))

        # Need-ordered input DMA: the simulated DMA engine drains transfers
        # roughly in issue order, so W0 and the first tokens go first, then
        # the recurrence weights, then the sequence bulk.
        sq, sc = nc.sync, nc.scalar
        sq.dma_start(out=w0[:, :, 0:2, :], in_=w0_d[:, :, 0:2, :])
        sc.dma_start(out=xT[:, :, 0:PTOK], in_=xT_d[:, :, 0:PTOK])
        sq.dma_start(out=w0[:, :, 2:4, :], in_=w0_d[:, :, 2:4, :])
        sc.dma_start(out=w0[:, :, 4:6, :], in_=w0_d[:, :, 4:6, :])
        sq.dma_start(out=w0[:, :, 6:8, :], in_=w0_d[:, :, 6:8, :])
        sc.dma_start(out=u0[:], in_=u0_d[:])
        sq.dma_start(out=xT[:, :, PTOK:2 * PTOK], in_=xT_d[:, :, PTOK:2 * PTOK])
        if with_b1:
            sc.dma_start(out=b1sq[:], in_=b1_d[:])
            sq.dma_start(out=indic[:], in_=in_d[:])
        sc.dma_start(out=w1[:], in_=w1_d[:])
        sq.dma_start(out=u1[:], in_=u1_d[:])
        sc.dma_start(out=wd[:], in_=wd_d[:])
        sq.dma_start(out=bds[:], in_=bd_d[:])
        for i, (ca, cb) in enumerate(((2, 4), (4, 6), (6, 8))):
            (sc if i % 2 == 0 else sq).dma_start(
                out=xT[:, :, ca * PTOK:cb * PTOK],
                in_=xT_d[:, :, ca * PTOK:cb * PTOK])

        def p1_quarter_tile(q):
            return zqp.tile([128, GC, QS, BL], F32, tag="zq", name=f"zq_{q}")

        def p1_slice(q, zq, g_list, first, close=False):
            # z0 step-bank gate-groups: 8 K-matmuls each, straight into PSUM.
            # Only the step's very first matmul carries start=True (it marks
            # the whole bank pending-zero; later matmuls overwrite on first
            # touch and accumulate after) -- one group per bank, closed by
            # the step's last U matmul (or here, for step 0).
            for i, g in enumerate(g_list):
                for k in range(KT_IN):
                    nc.tensor.matmul(
                        zq[:, g, :, :], w0[:, k, g, :],
                        xT[:, k, q * QTOK:(q + 1) * QTOK],
                        start=(first and i == 0 and k == 0),
                        stop=(close and i == len(g_list) - 1
                              and k == KT_IN - 1))

        def p5_mm(m, s0, ns):
            # out[:, m, steps s0:s0+ns] = Wd[:, m].T @ h1 (+ bd[m] at copy-out)
            ps = p5p.tile([128, PTOK], F32, tag="zq", name=f"p5_{m}_{s0}")
            ps = ps[:, 0:ns * BL]
            for k in range(HC):
                nc.tensor.matmul(
                    ps[:], wd[:, k, m, :], h1[:, s0:s0 + ns, k, :],
                    start=(k == 0), stop=(k == HC - 1))
            return ps

        def p5_out(m, s0, ns, ps, par, after=None, ot=None, dma=True):
            if ot is None:
                ot = outp.tile([128, PTOK], F32, tag="ot", name=f"ot{m}_{s0}")
                ot = ot[:, 0:ns * BL]
            # gap-filler priority class: the scheduler slots these only
            # when the engine has nothing critical ready, so the copy lands
            # in post-cell idle time instead of splitting the cell chain
            with tc.high_priority(offset=-1_000_000):
                if par % 2 == 0:
                    nc.scalar.activation(ot[:], ps[:], AF.Identity,
                                         bias=bds[:, m:m + 1])
                else:
                    nc.vector.tensor_scalar(ot[:], ps[:], bds[:, m:m + 1],
                                            None, op0=ALU.add)
            # out-DMA always issues from SP: its descriptor-gen would steal
            # ~0.6us of ACT SEQ time per issue from the sigmoid stream
            if dma:
                sq.dma_start(out=out_d[:, m, s0 * BL:(s0 + ns) * BL], in_=ot[:])

        st0 = _L0Stepper(nc, work, u0, h0, c0)
        st1 = _L1Stepper(nc, work, ps_r, u1, w1, h0, h1, c1,
                         b1sq=b1sq if with_b1 else None,
                         indic=indic if with_b1 else None)

        # step-banks 0..2 run under the weight DMA, before round 0
        zq_tiles = {}
        for q in (0, 1, 2):
            zq_tiles[q] = p1_quarter_tile(q)
            p1_slice(q, zq_tiles[q], range(GC if q < 2 else GC // 2),
                     True, close=(q == 0))

        # Wavefront: L1 trails L0 by LAG steps. One future p1 quarter-slice
        # (16 matmuls = 2 gate-groups) is injected per round so quarter q is
        # complete just before round 4q; the output m-tiles of a finished h1
        # chunk are spread over the following rounds.
        LAG = 2
        pend = []     # p5 copies deferred past the round's cell chain
        for t in range(T + LAG):
            j = t - LAG
            a = t if t < T else None
            b = j if 0 <= j < T else None
            if a is not None:
                st0.set_quarter(zq_tiles.pop(a))
                st0.emit_mm(a)
            if b is not None:
                st1.emit_mm(b)
            if a is not None and a + 2 < T:
                # finish step a+2's bank (gates 4-7)
                p1_slice(a + 2, zq_tiles[a + 2], range(4, GC), False)
            if a is not None and a + 3 < T:
                # open step a+3's bank (gates 0-3)
                q = a + 3
                zq_tiles[q] = p1_quarter_tile(q)
                p1_slice(q, zq_tiles[q], range(4), True)
            if b is not None and b >= PCH and b % PCH < MT and b < (NPCH - 1) * PCH + MT:
                cix, m = b // PCH - 1, b % PCH
                pend.append((m, cix * PCH, PCH,
                             p5_mm(m, cix * PCH, PCH), m + cix))
            if b is not None and b >= (NPCH - 1) * PCH + 4:
                # last chunk, first half (steps 56-59): ready after b=59;
                # spread its m-tiles over rounds b=60..63
                ms = [b - ((NPCH - 1) * PCH + 4)]
                if b == T - 1:
                    ms.append(MT - 1)
                for mm_ in ms:
                    pend.append((mm_, (NPCH - 1) * PCH, PCH // 2,
                                 p5_mm(mm_, (NPCH - 1) * PCH, PCH // 2), mm_))
            if a is not None:
                st0.emit_sig(a)
            if b is not None:
                st1.emit_sig(b)
            if a is not None:
                st0.emit_cell(a)
            if b is not None:
                st1.emit_cell(b)
            # flush copies whose matmul ran LAST round: their data has been
            # ready for a full round, so wherever the in-order streams place
            # them they never block a fresher op
            while len(pend) > 1 or (t == T + LAG - 1 and pend):
                p5_out(*pend.pop(0))
        # tail: last half-chunk, batched into one SBUF tile and one DMA
        tail_ps = [p5_mm(m, T - PCH // 2, PCH // 2) for m in range(2)]
        tail_ps += [zqp.tile([128, GC, QS, BL], F32, tag="zq",
                             name=f"p5t_{m}").rearrange(
                                 "p a b c -> p (a b c)")[:, 0:PTOK // 2]
                    for m in range(2, MT)]
        for m in range(2, MT):
            ps = tail_ps[m]
            for k in range(HC):
                nc.tensor.matmul(
                    ps[:], wd[:, k, m, :], h1[:, T - PCH // 2:T, k, :],
                    start=(k == 0), stop=(k == HC - 1))
        tot = outp.tile([128, MT, PTOK // 2], F32, tag="tot", name="tail_ot")
        for m in range(MT):
            p5_out(m, T - PCH // 2, PCH // 2, tail_ps[m], m,
                   ot=tot[:, m, :], dma=False)
        sq.dma_start(out=out_d[:, :, NTOK - PTOK // 2:NTOK], in_=tot[:])
    _dedup_ldweights(nc)
    nc.compile()
    return nc


def _dedup_ldweights(nc):
    """Drop an Ldweights whose stationary operand is identical to the
    immediately-preceding PE weight load -- the array still holds it, so the
    following (non-self-loading) Matmult reuses the loaded weights."""
    removed = 0
    for blk in nc.m.functions[0].blocks:
        insts = blk.instructions
        out = []
        last_w = None
        for inst in insts:
            if inst.opcode == "Ldweights":
                si = inst.sync_info
                key = repr(inst.ins[0])
                if key == last_w and (si is None or not si.on_wait):
                    removed += 1
                    continue
                last_w = key
            elif inst.opcode != "Matmult" and inst.engine == mybir.EngineType.PE:
                last_w = None
            out.append(inst)
        if removed:
            blk.instructions.clear()
            blk.instructions.extend(out)
    return removed


_NC_CACHE = {}
LAST_RESULTS = []  # test harness introspection (exec_time_ns / traces)


def _get_nc(with_b1=False):
    key = ("nc", with_b1)
    if key not in _NC_CACHE:
        _NC_CACHE[key] = build_nc(with_b1=with_b1)
    return _NC_CACHE[key]


def kernel(**inputs):
    x = np.asarray(inputs["x"], np.float32)
    b1 = np.asarray(inputs["b1"], np.float32)
    with_b1 = bool(np.any(b1))
    shared = _prep_shared(
        np.asarray(inputs["W0"], np.float32), np.asarray(inputs["U0"], np.float32),
        np.asarray(inputs["b0"], np.float32), np.asarray(inputs["W1"], np.float32),
        np.asarray(inputs["U1"], np.float32), b1,
        np.asarray(inputs["Wd"], np.float32), np.asarray(inputs["bd"], np.float32))
    in_maps = []
    for c in range(NCORES):
        m = dict(shared)
        m["xT"] = _prep_x(x[c * BL:(c + 1) * BL])
        in_maps.append(m)

    nc = _get_nc(with_b1=with_b1)
    res = bass_utils.run_bass_kernel_spmd(nc, in_maps, core_ids=list(range(NCORES)))
    LAST_RESULTS.append(res)

    outs = []
    for c in range(NCORES):
        o = np.asarray(res.results[c]["out"], np.float32)      # (128, MT, NTOK)
        yT = o.transpose(1, 0, 2).reshape(D_OUT_PAD, NTOK)[:D_OUT]
        y = yT.T.reshape(T, BL, N, F_OUT).transpose(1, 2, 0, 3)
        outs.append(y)
    return np.ascontiguousarray(np.concatenate(outs, axis=0), dtype=np.float32)


# revision 49
# speedup vs baseline: 1.2688x; 1.0541x over previous
"""Bass/Tile Trainium2 kernel for the 2-layer FC-LSTM + Dense model.

Strategy (data-parallel over batch, 8 cores x 32 samples):
  - All on-chip activations live in "transposed" layout: feature dim on the
    128 SBUF partitions, (time x batch) on the free dim.
  - Gate columns of W/U are host-permuted from keras order [i f g o] to
    [i f o g] so one sigmoid covers a contiguous 6-chunk block.
  - The input GEMM writes z0 = W0.T x + b0 DIRECTLY into the recurrence
    PSUM banks, one bank per timestep (one accumulation group per bank:
    opened by the step's first input matmul, closed by its last U matmul).
    The gate math reads PSUM in place: no identity prefill, no PSUM
    evacuation, no zx staging in SBUF.
  - b0 is folded into the input GEMM via a ones-row in the K-padded input.
    b1/bd keep a generic path, but when they are all-zero (as produced by
    setup_inputs) a fast path drops the L1 bias matmul.
  - Input-GEMM slices for step t+3 are injected into round t so there is
    no serial GEMM prologue; PE is warmed with dummy matmuls
    during the initial weight DMA; the first ACT op is a sigmoid so a
    single table-set load covers every later activation/copy.
  - c(t) = f*c + i*relu(g) is nonnegative by induction (c0=0), so
    h = o*relu(c) == o*c and the h-write is a plain tensor_tensor multiply.
  - The dense-output copies are gated on the round's fresh h value so the
    scheduler cannot insert them into the latency-critical cell chain.
  - Matmuls in bf16 (fp32 PSUM accumulation); cell state, gate outputs and
    elementwise cell math in bf16 (2x DVE mode), z and dense output in f32.
"""

import numpy as np
import ml_dtypes
from contextlib import ExitStack

import concourse.mybir as mybir
import concourse.tile as tile
from concourse import bacc, bass_utils

# problem constants (hardcoded per contract)
B, N, T, F_IN = 256, 300, 64, 3
U_DIM = 256
G = 4 * U_DIM              # 1024 gates per layer
F_OUT = 2
D_IN = N * F_IN            # 900
D_OUT = N * F_OUT          # 600
NCORES = 8
BL = B // NCORES           # 32 batch rows per core
NTOK = T * BL              # 2048 tokens per core (token id = t*BL + b)
KP = 1024                  # padded input-feature dim; row 900 is the bias row
KT_IN = KP // 128          # 8 k-tiles for the input GEMM
GC = G // 128              # 8 gate chunks
HC = U_DIM // 128          # 2 hidden chunks
MT = 5                     # output m-tiles (600 -> 640)
D_OUT_PAD = MT * 128
QS = 1                     # steps per z0 PSUM bank (one group per bank)
QTOK = QS * BL             # 32 tokens per step-bank
NQ = T // QS               # one z0 bank per step
PCH = 8                    # steps per output-GEMM chunk
PTOK = PCH * BL            # 256 tokens
NPCH = T // PCH            # 8 chunks
NWARM = 16                 # PE warm-up matmuls (N=256)

BF16 = mybir.dt.bfloat16
F32 = mybir.dt.float32
NP_BF16 = ml_dtypes.bfloat16
AF = mybir.ActivationFunctionType
ALU = mybir.AluOpType

# keras gate order i,f,g,o -> our chunk order i,f,o,g
_PERM = np.concatenate([
    np.arange(0, U_DIM),                  # i
    np.arange(U_DIM, 2 * U_DIM),          # f
    np.arange(3 * U_DIM, 4 * U_DIM),      # o
    np.arange(2 * U_DIM, 3 * U_DIM),      # g
])


def _w_tiles(Wp, kt):
    """(kt*128, GC*128) f32 -> (128, kt, GC, 128) bf16 lhsT tile array."""
    return np.ascontiguousarray(
        Wp.astype(NP_BF16).reshape(kt, 128, GC, 128).transpose(1, 0, 2, 3))


def _prep_shared(W0, U0, b0, W1, U1, b1, Wd, bd):
    W0p = np.zeros((KP, G), np.float32)
    W0p[:D_IN] = W0[:, _PERM]
    W0p[D_IN] = b0[_PERM]
    w0t = _w_tiles(W0p, KT_IN)
    u0t = _w_tiles(U0[:, _PERM], HC)
    w1t = _w_tiles(W1[:, _PERM], HC)
    u1t = _w_tiles(U1[:, _PERM], HC)
    Wdp = np.zeros((U_DIM, D_OUT_PAD), np.float32)
    Wdp[:, :D_OUT] = Wd
    wdt = np.ascontiguousarray(
        Wdp.astype(NP_BF16).reshape(HC, 128, MT, 128).transpose(1, 0, 2, 3))
    # L1 bias via one matmul (generic path): stationary rows 0..7 hold b1
    # reshaped (8, 128); the moving "indicator" is 1.0 at (row g, col-block g).
    b1sq = np.zeros((128, 128), np.float32)
    b1sq[:GC] = b1[_PERM].reshape(GC, 128)
    b1sq = b1sq.astype(NP_BF16)
    indic = np.zeros((128, GC, BL), np.float32)
    for g in range(GC):
        indic[g, g, :] = 1.0
    indic = np.ascontiguousarray(indic.reshape(128, GC * BL).astype(NP_BF16))
    bdp = np.zeros(D_OUT_PAD, np.float32)
    bdp[:D_OUT] = bd
    bdt = np.ascontiguousarray(bdp.reshape(MT, 128).T)
    return dict(w0t=w0t, u0t=u0t, w1t=w1t, u1t=u1t, wdt=wdt, b1sq=b1sq,
                indic=indic, bdt=bdt)


def _prep_x(x_core):
    """(BL, N, T, F_IN) f32 -> (128, KT_IN, NTOK) bf16 with bias ones-row."""
    seq = x_core.transpose(0, 2, 1, 3).reshape(BL, T, D_IN)   # (b, t, feat)
    xT = np.zeros((KP, T, BL), np.float32)
    xT[:D_IN] = seq.transpose(2, 1, 0)                        # (feat, t, b)
    xT[D_IN] = 1.0
    return np.ascontiguousarray(
        xT.astype(NP_BF16).reshape(KT_IN, 128, NTOK).transpose(1, 0, 2))


class _L0Stepper:
    """L0: z lives in the p1 quarter PSUM tile; U matmuls accumulate in
    place and the gate math reads the step's slice directly."""

    def __init__(self, nc, work, u, hseq, cst):
        self.nc, self.work = nc, work
        self.u, self.hseq, self.cst = u, hseq, cst
        self.zq = None     # current quarter tile, set by the build loop
        self.ps = None     # [128, GC, BL] view of step slice
        self.sfo = None

    def set_quarter(self, zq):
        self.zq = zq

    def emit_mm(self, t):
        nc = self.nc
        self.ps = self.zq[:, :, 0, :]
        if t == 0:
            return
        for g in (6, 7, 0, 1, 2, 3, 4, 5):
            for k in range(HC):
                # the bank's accumulation group was opened by the step's
                # first p1 matmul; the last U matmul closes it
                nc.tensor.matmul(
                    self.zq[:, g, 0, :], self.u[:, k, g, :],
                    self.hseq[:, t - 1, k, :],
                    start=False, stop=(g == 5 and k == HC - 1))

    def emit_sig(self, t):
        sfo = self.work.tile([128, 6, BL], BF16, tag="l0sfo", name=f"l0_sfo_{t}")
        self.nc.scalar.activation(sfo[:], self.ps[:, 0:6, :], AF.Sigmoid)
        self.sfo = sfo

    def emit_cell(self, t):
        nc, sfo, cst = self.nc, self.sfo, self.cst
        if t == 0:
            nc.vector.scalar_tensor_tensor(
                cst[:], self.ps[:, 6:8, :], 0.0, sfo[:, 0:2, :],
                op0=ALU.max, op1=ALU.mult)
        else:
            t1 = self.work.tile([128, 2, BL], BF16, tag="l0t1", name=f"l0_t1_{t}")
            nc.vector.scalar_tensor_tensor(
                t1[:], self.ps[:, 6:8, :], 0.0, sfo[:, 0:2, :],
                op0=ALU.max, op1=ALU.mult)
            nc.vector.tensor_mul(cst[:], cst[:], sfo[:, 2:4, :])
            nc.vector.tensor_add(cst[:], cst[:], t1[:])
        # c >= 0 by induction, so h = o * relu(c) == o * c
        return nc.vector.tensor_mul(self.hseq[:, t, :, :], cst[:], sfo[:, 4:6, :])


class _L1Stepper:
    """L1: z(t) = [b1 +] W1.T h0(t) + U1.T h1(t-1), built per step in its
    own PSUM bank."""

    def __init__(self, nc, work, ps_r, u, w_in, hprev, hseq, cst,
                 b1sq=None, indic=None):
        self.nc, self.work, self.ps_r = nc, work, ps_r
        self.u, self.w_in, self.hprev = u, w_in, hprev
        self.hseq, self.cst = hseq, cst
        self.b1sq, self.indic = b1sq, indic
        self.ps = self.sfo = None

    def emit_mm(self, t):
        nc = self.nc
        ps = self.ps_r.tile([128, GC, BL], F32, tag="l1ps",
                            name=f"l1_ps_{t}", bufs=2)
        self.ps = ps
        last_in = t == 0
        if self.b1sq is not None:
            nc.tensor.matmul(ps[:], self.b1sq[:], self.indic[:],
                             start=True, stop=False)
        for g in range(GC):
            for k in range(HC):
                nc.tensor.matmul(
                    ps[:, g, :], self.w_in[:, k, g, :], self.hprev[:, t, k, :],
                    start=(self.b1sq is None and g == 0 and k == 0),
                    stop=(g == GC - 1 and k == HC - 1 and last_in))
        if t == 0:
            return
        for g in (6, 7, 0, 1, 2, 3, 4, 5):
            for k in range(HC):
                nc.tensor.matmul(
                    ps[:, g, :], self.u[:, k, g, :], self.hseq[:, t - 1, k, :],
                    start=False, stop=(g == 5 and k == HC - 1))

    def emit_sig(self, t):
        sfo = self.work.tile([128, 6, BL], BF16, tag="l1sfo", name=f"l1_sfo_{t}")
        self.nc.scalar.activation(sfo[:], self.ps[:, 0:6, :], AF.Sigmoid)
        self.sfo = sfo

    def emit_cell(self, t):
        nc, sfo, cst = self.nc, self.sfo, self.cst
        if t == 0:
            nc.vector.scalar_tensor_tensor(
                cst[:], self.ps[:, 6:8, :], 0.0, sfo[:, 0:2, :],
                op0=ALU.max, op1=ALU.mult)
        else:
            t1 = self.work.tile([128, 2, BL], BF16, tag="l1t1", name=f"l1_t1_{t}")
            nc.vector.scalar_tensor_tensor(
                t1[:], self.ps[:, 6:8, :], 0.0, sfo[:, 0:2, :],
                op0=ALU.max, op1=ALU.mult)
            nc.vector.tensor_mul(cst[:], cst[:], sfo[:, 2:4, :])
            nc.vector.tensor_add(cst[:], cst[:], t1[:])
        return nc.vector.tensor_mul(self.hseq[:, t, :, :], cst[:], sfo[:, 4:6, :])


def build_nc(with_b1=False):
    nc = bacc.Bacc("TRN2", target_bir_lowering=False, debug=False)
    xT_d = nc.dram_tensor("xT", (128, KT_IN, NTOK), BF16, kind="ExternalInput").ap()
    w0_d = nc.dram_tensor("w0t", (128, KT_IN, GC, 128), BF16, kind="ExternalInput").ap()
    u0_d = nc.dram_tensor("u0t", (128, HC, GC, 128), BF16, kind="ExternalInput").ap()
    w1_d = nc.dram_tensor("w1t", (128, HC, GC, 128), BF16, kind="ExternalInput").ap()
    u1_d = nc.dram_tensor("u1t", (128, HC, GC, 128), BF16, kind="ExternalInput").ap()
    wd_d = nc.dram_tensor("wdt", (128, HC, MT, 128), BF16, kind="ExternalInput").ap()
    b1_d = nc.dram_tensor("b1sq", (128, 128), BF16, kind="ExternalInput").ap()
    in_d = nc.dram_tensor("indic", (128, GC * BL), BF16, kind="ExternalInput").ap()
    bd_d = nc.dram_tensor("bdt", (128, MT), F32, kind="ExternalInput").ap()
    out_d = nc.dram_tensor("out", (128, MT, NTOK), F32, kind="ExternalOutput").ap()

    with tile.TileContext(nc) as tc, ExitStack() as ctx:
        const = ctx.enter_context(tc.tile_pool(name="const", bufs=1))
        xT = const.tile([128, KT_IN, NTOK], BF16)
        w0 = const.tile([128, KT_IN, GC, 128], BF16)
        u0 = const.tile([128, HC, GC, 128], BF16)
        w1 = const.tile([128, HC, GC, 128], BF16)
        u1 = const.tile([128, HC, GC, 128], BF16)
        wd = const.tile([128, HC, MT, 128], BF16)
        bds = const.tile([128, MT], F32)
        h0 = const.tile([128, T, HC, BL], BF16)
        h1 = const.tile([128, T, HC, BL], BF16)
        c0 = const.tile([128, HC, BL], BF16)
        c1 = const.tile([128, HC, BL], BF16)
        warm = const.tile([128, 2, 128], BF16)
        junk = const.tile([128, 8], F32)
        if with_b1:
            b1sq = const.tile([128, 128], BF16)
            indic = const.tile([128, GC * BL], BF16)

        # PSUM: z0 step-banks and p5 share one 6-slot pool; L1 z 2 -> 8 banks
        zqp = ctx.enter_context(tc.tile_pool(name="zqp", bufs=6, space="PSUM"))
        ps_r = ctx.enter_context(tc.tile_pool(name="ps_r", bufs=2, space="PSUM"))
        p5p = zqp
        work = ctx.enter_context(tc.tile_pool(name="work", bufs=3))
        outp = ctx.enter_context(tc.tile_pool(name="outp", bufs=3))

        # --- prologue: ACT table preload, PE warm-up, need-ordered DMA ---
        nc.gpsimd.memset(warm[:], 0)
        nc.gpsimd.memset(junk[:], 0)
        # first ACT op is a Sigmoid: the single table-set load covers the
        # later identity copies too (sigmoid_and_others has copy/relu).
        nc.scalar.activation(junk[:], junk[:], AF.Sigmoid)
        # dummy matmuls ramp the PE p-state while the first DMAs land
        wps = p5p.tile([128, 256], F32, tag="zq", name="warmps")
        for i in range(NWARM):
            nc.tensor.matmul(wps[:], warm[:, 0, :], warm.rearrange("p a b -> p (a b)"),
                             start=(i == 0), stop=(i == NWARM - 1))

        # Need-ordered input DMA: the simulated DMA engine drains transfers
        # roughly in issue order, so W0 and the first tokens go first, then
        # the recurrence weights, then the sequence bulk.
        sq, sc = nc.sync, nc.scalar
        sq.dma_start(out=w0[:, :, 0:2, :], in_=w0_d[:, :, 0:2, :])
        sc.dma_start(out=xT[:, :, 0:256], in_=xT_d[:, :, 0:256])
        sq.dma_start(out=w0[:, :, 2:4, :], in_=w0_d[:, :, 2:4, :])
        sc.dma_start(out=w0[:, :, 4:6, :], in_=w0_d[:, :, 4:6, :])
        sc.dma_start(out=u0[:], in_=u0_d[:])
        sq.dma_start(out=w0[:, :, 6:8, :], in_=w0_d[:, :, 6:8, :])
        sq.dma_start(out=xT[:, :, 256:512], in_=xT_d[:, :, 256:512])
        if with_b1:
            sc.dma_start(out=b1sq[:], in_=b1_d[:])
            sq.dma_start(out=indic[:], in_=in_d[:])
        sc.dma_start(out=w1[:], in_=w1_d[:])
        sq.dma_start(out=u1[:], in_=u1_d[:])
        sc.dma_start(out=wd[:], in_=wd_d[:])
        sq.dma_start(out=bds[:], in_=bd_d[:])
        for i, (ca, cb) in enumerate(((512, 1024), (1024, 1536), (1536, 2048))):
            (sc if i % 2 == 0 else sq).dma_start(
                out=xT[:, :, ca:cb], in_=xT_d[:, :, ca:cb])

        def p1_quarter_tile(q):
            return zqp.tile([128, GC, QS, BL], F32, tag="zq", name=f"zq_{q}")

        def p1_slice(q, zq, g_list, first, close=False):
            # z0 step-bank gate-groups: 8 K-matmuls each, straight into PSUM.
            # Only the step's very first matmul carries start=True (it marks
            # the whole bank pending-zero; later matmuls overwrite on first
            # touch and accumulate after) -- one group per bank, closed by
            # the step's last U matmul (or here, for step 0).
            for i, g in enumerate(g_list):
                for k in range(KT_IN):
                    nc.tensor.matmul(
                        zq[:, g, :, :], w0[:, k, g, :],
                        xT[:, k, q * QTOK:(q + 1) * QTOK],
                        start=(first and i == 0 and k == 0),
                        stop=(close and i == len(g_list) - 1
                              and k == KT_IN - 1))

        def p5_mm(m, s0, ns):
            # out[:, m, steps s0:s0+ns] = Wd[:, m].T @ h1 (+ bd[m] at copy-out)
            ps = p5p.tile([128, PTOK], F32, tag="zq", name=f"p5_{m}_{s0}")
            ps = ps[:, 0:ns * BL]
            for k in range(HC):
                nc.tensor.matmul(
                    ps[:], wd[:, k, m, :], h1[:, s0:s0 + ns, k, :],
                    start=(k == 0), stop=(k == HC - 1))
            return ps

        def p5_out(m, s0, ns, ps, par, gate=None, ot=None, dma=True):
            if ot is None:
                ot = outp.tile([128, PTOK], F32, tag="ot", name=f"ot{m}_{s0}")
                ot = ot[:, 0:ns * BL]
            bias = bds[:, m:m + 1]
            if gate is not None:
                # thread the bias through a tile computed from the freshest
                # h value: the copy then has a true data dep on the round's
                # cell chain and cannot be greedily inserted into it
                bg = work.tile([128, 1], F32, tag="bg", name=f"bg{m}_{s0}")
                nc.vector.scalar_tensor_tensor(
                    bg[:], gate, 0.0, bias, op0=ALU.mult, op1=ALU.add)
                bias = bg[:]
            if par % 2 == 0:
                nc.scalar.activation(ot[:], ps[:], AF.Identity, bias=bias)
            else:
                nc.vector.tensor_scalar(ot[:], ps[:], bias, None, op0=ALU.add)
            # out-DMA always issues from SP: its descriptor-gen would steal
            # ~0.6us of ACT SEQ time per issue from the sigmoid stream
            if dma:
                sq.dma_start(out=out_d[:, m, s0 * BL:(s0 + ns) * BL], in_=ot[:])

        st0 = _L0Stepper(nc, work, u0, h0, c0)
        st1 = _L1Stepper(nc, work, ps_r, u1, w1, h0, h1, c1,
                         b1sq=b1sq if with_b1 else None,
                         indic=indic if with_b1 else None)

        # step-banks 0..3 run under the weight DMA, before round 0
        zq_tiles = {}
        for q in (0, 1, 2, 3):
            zq_tiles[q] = p1_quarter_tile(q)
            p1_slice(q, zq_tiles[q], range(GC if q < 3 else GC // 2),
                     True, close=(q == 0))

        # Wavefront: L1 trails L0 by LAG steps. One future p1 quarter-slice
        # (16 matmuls = 2 gate-groups) is injected per round so quarter q is
        # complete just before round 4q; the output m-tiles of a finished h1
        # chunk are spread over the following rounds.
        LAG = 3
        pend = []     # p5 copies deferred past the round's cell chain
        for t in range(T + LAG):
            j = t - LAG
            a = t if t < T else None
            b = j if 0 <= j < T else None
            if a is not None:
                st0.set_quarter(zq_tiles.pop(a))
                st0.emit_mm(a)
            if b is not None:
                st1.emit_mm(b)
            if a is not None and a + 3 < T:
                # finish step a+3's bank (gates 4-7)
                p1_slice(a + 3, zq_tiles[a + 3], range(4, GC), False)
            if a is not None and a + 4 < T:
                # open step a+4's bank (gates 0-3)
                q = a + 4
                zq_tiles[q] = p1_quarter_tile(q)
                p1_slice(q, zq_tiles[q], range(4), True)
            if b is not None and b >= PCH and b % PCH < MT and b < (NPCH - 1) * PCH + MT:
                cix, m = b // PCH - 1, b % PCH
                pend.append((m, cix * PCH, PCH,
                             p5_mm(m, cix * PCH, PCH), m + cix))
            if b is not None and b >= (NPCH - 1) * PCH + PCH // 2:
                # last chunk, first half: spread its m-tiles over the
                # remaining rounds
                ms = [b - ((NPCH - 1) * PCH + PCH // 2)]
                if b == T - 1:
                    ms.extend(range(ms[0] + 1, MT))
                for mm_ in ms:
                    if mm_ < MT:
                        pend.append((mm_, (NPCH - 1) * PCH, PCH // 2,
                                     p5_mm(mm_, (NPCH - 1) * PCH, PCH // 2),
                                     mm_))
            if a is not None:
                st0.emit_sig(a)
            if b is not None:
                st1.emit_sig(b)
            if a is not None:
                st0.emit_cell(a)
            if b is not None:
                st1.emit_cell(b)
            # flush copies whose matmul ran LAST round, gated on this
            # round's last h write so they land after the cell chain
            g8 = h1[:, b, 0, 0:1] if b is not None else h0[:, a, 0, 0:1]
            while len(pend) > 1 or (t == T + LAG - 1 and pend):
                p5_out(*pend.pop(0), gate=g8)
        # tail: last half-chunk, batched into one SBUF tile and one DMA
        tail_ps = [p5_mm(m, T - PCH // 2, PCH // 2) for m in range(2)]
        tail_ps += [zqp.tile([128, GC, QS, BL], F32, tag="zq",
                             name=f"p5t_{m}").rearrange(
                                 "p a b c -> p (a b c)")[:, 0:PTOK // 2]
                    for m in range(2, MT)]
        for m in range(2, MT):
            ps = tail_ps[m]
            for k in range(HC):
                nc.tensor.matmul(
                    ps[:], wd[:, k, m, :], h1[:, T - PCH // 2:T, k, :],
                    start=(k == 0), stop=(k == HC - 1))
        tot = outp.tile([128, MT, PTOK // 2], F32, tag="tot", name="tail_ot")
        for m in range(MT):
            p5_out(m, T - PCH // 2, PCH // 2, tail_ps[m], m,
                   ot=tot[:, m, :], dma=False)
        sq.dma_start(out=out_d[:, :, NTOK - PTOK // 2:NTOK], in_=tot[:])
    _dedup_ldweights(nc)
    nc.compile()
    return nc


def _dedup_ldweights(nc):
    """Drop an Ldweights whose stationary operand is identical to the
    immediately-preceding PE weight load -- the array still holds it, so the
    following (non-self-loading) Matmult reuses the loaded weights."""
    removed = 0
    for blk in nc.m.functions[0].blocks:
        insts = blk.instructions
        out = []
        last_w = None
        for inst in insts:
            if inst.opcode == "Ldweights":
                si = inst.sync_info
                key = repr(inst.ins[0])
                if key == last_w and (si is None or not si.on_wait):
                    removed += 1
                    continue
                last_w = key
            elif inst.opcode != "Matmult" and inst.engine == mybir.EngineType.PE:
                last_w = None
            out.append(inst)
        if removed:
            blk.instructions.clear()
            blk.instructions.extend(out)
    return removed


_NC_CACHE = {}
LAST_RESULTS = []  # test harness introspection (exec_time_ns / traces)


def _get_nc(with_b1=False):
    key = ("nc", with_b1)
    if key not in _NC_CACHE:
        _NC_CACHE[key] = build_nc(with_b1=with_b1)
    return _NC_CACHE[key]


def kernel(**inputs):
    x = np.asarray(inputs["x"], np.float32)
    b1 = np.asarray(inputs["b1"], np.float32)
    with_b1 = bool(np.any(b1))
    shared = _prep_shared(
        np.asarray(inputs["W0"], np.float32), np.asarray(inputs["U0"], np.float32),
        np.asarray(inputs["b0"], np.float32), np.asarray(inputs["W1"], np.float32),
        np.asarray(inputs["U1"], np.float32), b1,
        np.asarray(inputs["Wd"], np.float32), np.asarray(inputs["bd"], np.float32))
    in_maps = []
    for c in range(NCORES):
        m = dict(shared)
        m["xT"] = _prep_x(x[c * BL:(c + 1) * BL])
        in_maps.append(m)

    nc = _get_nc(with_b1=with_b1)
    res = bass_utils.run_bass_kernel_spmd(nc, in_maps, core_ids=list(range(NCORES)))
    LAST_RESULTS.append(res)

    outs = []
    for c in range(NCORES):
        o = np.asarray(res.results[c]["out"], np.float32)      # (128, MT, NTOK)
        yT = o.transpose(1, 0, 2).reshape(D_OUT_PAD, NTOK)[:D_OUT]
        y = yT.T.reshape(T, BL, N, F_OUT).transpose(1, 2, 0, 3)
        outs.append(y)
    return np.ascontiguousarray(np.concatenate(outs, axis=0), dtype=np.float32)
